# revision 54
# baseline (speedup 1.0000x reference)
"""Causal multi-head self-attention (b=4, s=2048, d_model=1024, 16 heads) on 8
Trainium2 NeuronCores.

Sharding: core c handles batch c//2 and head-group c%2 (8 of 16 heads):
wqkv row-split by head (tensor parallel), wo column-split; the host sums the
two partials of each batch while unsharding.

Design (HW-measured 323 us vs the 465 us baseline this replaced; the exp()
stream on the scalar/ACT engine, ~158 us busy, is the floor everything else
must hide under):
  - All inputs SBUF-resident, loaded once at t=0 across three DMA rings
    (sync: x chunk-major; gpsimd: wqk by head-pair group; scalar: cos/sin
    chunks, tables, wv, wo). No per-phase reloads.
  - Projections are 8-matmul PSUM-accumulation "groups" (K=1024 via 8 eo
    slices, N=512): V token-major into vbuf (+ ones column via memset for
    free softmax denominators); Q/K feature-major with RoPE as cos-mult +
    sin-mult + full-row SWDGE swap-add (A0 uses per-tci swap-adds so C0
    starts early). Groups are hand-paced between attention k-tiles so the PE
    stays dense while ACT streams exps.
  - Attention per (head pair, q-chunk of 512) over causal k-tiles, software
    pipelined: 4 k-tiles of scores+exp lead their AV matmuls, and the
    previous chunk's denominator tail is emitted inside the next chunk, so
    the single-buffered AV PSUM never stalls the exp stream. Scores are a
    row-tiled matmul pair (heads at partition halves, causally N-trimmed),
    causal mask via ident@mtri accumulate, one exp per k-tile covering both
    heads' live columns, AV accumulates [V|1] so PSUM row 64 is the softmax
    denominator.
  - Denominator chain off the ACT engine: DVE drains PSUM row 64, two K=1
    one-hot matmuls broadcast both heads' denominators across partition
    halves (no partition-restack DMA), 1-pass DVE approx-reciprocal,
    normalize fused into the y^T PSUM drain.
  - Output projection pipelined into the last attention phase (PSUM bank
    freed by the projection pool funds double-buffering); bf16 partials
    DMA'd out on two rings, host sums core pairs (the TP all-reduce).
"""

import sys

if "/opt/trn_rl_repo" not in sys.path:
    sys.path.insert(0, "/opt/trn_rl_repo")

from contextlib import ExitStack

import numpy as np

import concourse.bass as bass  # noqa: F401
import concourse.tile as tile
from concourse import bacc, mybir
from concourse.bass_utils import run_bass_kernel_spmd

F32 = mybir.dt.float32
F32R = mybir.dt.float32r
BF16 = mybir.dt.bfloat16
EXP = mybir.ActivationFunctionType.Exp
MULT = mybir.AluOpType.mult
ADD = mybir.AluOpType.add

B, S, D = 4, 2048, 1024
NH_CORE = 8      # heads per core
DH = 64          # head dim
P = 128
TCH = 512        # q/t chunk size
N_HP = NH_CORE // 2
NEG = -1.0e30
ROPE_THETA = 10000.0
SCALE = 1.0 / 8.0  # 1/sqrt(DH)

_CACHE = {}


def _emit(nc, tc, xTp, wqkp, wvp, wop, cosp, sinp, mtri, ident, onesh, permP, outp):
    mm = nc.tensor.matmul
    n_kt = S // P  # 16

    with ExitStack() as ctx:
        # ---------------- persistent (resident) buffers ----------------
        persist = ctx.enter_context(tc.tile_pool(name="persist", bufs=1))
        # chunk-major so each load DMA writes contiguous 8KB/partition
        xT_sb = persist.tile([P, 4, 8, TCH], BF16, tag="xT", name="xT_sb")
        # head-pair-group major: g holds Q f-tile g (cols 0:128) + K f-tile
        # 4+g (cols 128:256), so each group loads in one contiguous DMA
        wqk_sb = persist.tile([P, 4, 8, 2 * P], BF16, tag="wqk", name="wqk_sb")
        wv_sb = persist.tile([P, 8, 512], BF16, tag="wv", name="wv_sb")
        wo_sb = persist.tile([P, 4, 1024], BF16, tag="wo", name="wo_sb")
        cos_sb = persist.tile([P, S], F32, tag="cos", name="cos_sb")
        sin_sb = persist.tile([P, S], F32, tag="sin", name="sin_sb")
        qkT = [
            persist.tile([P, S], BF16, tag=f"qkT{ft}", name=f"qkT{ft}")
            for ft in range(8)
        ]
        vbuf = persist.tile([P, n_kt, NH_CORE, DH + 1], BF16, tag="vbuf", name="vbuf")
        yT = [
            persist.tile([P, S], BF16, tag=f"yT{hp}", name=f"yT{hp}")
            for hp in range(N_HP)
        ]
        ident_sb = persist.tile([P, P], BF16, tag="ident", name="ident_sb")
        mtri_sb = persist.tile([P, P], BF16, tag="mtri", name="mtri_sb")
        onesh_sb = persist.tile([1, 2 * P], BF16, tag="onesh", name="onesh_sb")
        permP_sb = persist.tile([P, P], BF16, tag="permP", name="permP_sb")

        # ---------------- initial loads (3 rings in parallel) ----------------
        # gpsimd: first 2 wqk groups (A0 needs g0 only); sync: x chunks;
        # scalar: cos/sin chunks paced with A0's token chunks + the rest.
        nc.gpsimd.dma_start(wqk_sb[:, 0, :, :], wqkp[0].ap()[:, :, :])
        nc.gpsimd.dma_start(wqk_sb[:, 1, :, :], wqkp[1].ap()[:, :, :])
        for tc4 in range(4):
            nc.sync.dma_start(xT_sb[:, tc4, :, :], xTp[tc4].ap()[:, :, :])
        for tc4 in range(2):
            tsl = slice(tc4 * TCH, (tc4 + 1) * TCH)
            nc.scalar.dma_start(cos_sb[:, tsl], cosp.ap()[:, tsl])
            nc.scalar.dma_start(sin_sb[:, tsl], sinp.ap()[:, tsl])
        nc.scalar.dma_start(mtri_sb[:], mtri.ap()[:, :])
        nc.scalar.dma_start(ident_sb[:], ident.ap()[:, :])
        nc.scalar.dma_start(onesh_sb[:], onesh.ap()[:, :])
        nc.gpsimd.dma_start(permP_sb[:], permP.ap()[:, :])
        for tc4 in range(2, 4):
            tsl = slice(tc4 * TCH, (tc4 + 1) * TCH)
            nc.scalar.dma_start(cos_sb[:, tsl], cosp.ap()[:, tsl])
            nc.scalar.dma_start(sin_sb[:, tsl], sinp.ap()[:, tsl])
        nc.scalar.dma_start(wv_sb[:], wvp.ap()[:, :, :])
        nc.scalar.dma_start(wo_sb[:], wop.ap()[:, :, :])
        nc.vector.memset(vbuf[:, :, :, DH : DH + 1], 1.0)

        # ---------------- SBUF working pools ----------------
        btpool = ctx.enter_context(tc.tile_pool(name="btmp", bufs=3))
        epool = ctx.enter_context(tc.tile_pool(name="expS", bufs=12))
        dpool = ctx.enter_context(tc.tile_pool(name="denst", bufs=2))
        rpool = ctx.enter_context(tc.tile_pool(name="recb", bufs=2))

        # ---------------- projection group helpers ----------------
        def b_group(g, pj):
            # V projection for t-tile g (token-major), ones col via memset.
            flush_a()
            vps = pj.tile([P, TCH], F32, tag="pj", name="vps")
            tci, tt = divmod(g, 4)
            for ec in range(8):
                mm(
                    vps[:],
                    xT_sb[:, tci, ec, tt * P : (tt + 1) * P],
                    wv_sb[:, ec, :],
                    start=(ec == 0),
                    stop=(ec == 7),
                )
            nc.vector.tensor_copy(vbuf[:, g, :, 0:DH], vps[:])

        a_finish = []

        def flush_a():
            while a_finish:
                a_finish.pop(0)()

        bt_live = {}

        def a_group(ft, tci, pj, rope="dma"):
            # Q/K projection + rope for f-tile ft, token chunk tci.
            # rope="perm": qkT = cos*ps + P @ (sin_pm*ps) with the 32<->32
            #   row-swap done by one PE matmul (constant permutation lhsT);
            #   the perm matmul + add of the PREVIOUS group are emitted after
            #   this group's matmuls so the PE never waits a sin-mult in line.
            #   Used for A0, which gates the first exp (gpsimd ring too slow).
            # rope="dma": sin-products accumulate into a full-row btf and 4
            #   SWDGE swap-adds run per f-tile after the last chunk (the ring
            #   is idle mid-kernel; 4KB descriptors amortize best).
            tsl = slice(tci * TCH, (tci + 1) * TCH)
            g, half = ft % 4, ft // 4
            ps = pj.tile([P, TCH], F32, tag="pj", name="ps")
            for ec in range(8):
                mm(
                    ps[:],
                    wqk_sb[:, g, ec, half * P : (half + 1) * P],
                    xT_sb[:, tci, ec, :],
                    start=(ec == 0),
                    stop=(ec == 7),
                )
            flush_a()
            if rope == "perm":
                btf = btpool.tile([P, TCH], BF16, tag="bt", name="btf")
                ctf = btpool.tile([P, TCH], BF16, tag="ct", name="ctf")
                nc.vector.tensor_tensor(btf[:], ps[:], sin_sb[:, tsl], MULT)
                nc.vector.tensor_tensor(ctf[:], ps[:], cos_sb[:, tsl], MULT)

                def finish():
                    ps2 = pj.tile([P, TCH], F32, tag="pj", name="ps2")
                    mm(ps2[:], permP_sb[:], btf[:], start=True, stop=True)
                    nc.vector.tensor_tensor(qkT[ft][:, tsl], ctf[:], ps2[:], ADD)

                a_finish.append(finish)
            elif rope == "dma_tci":
                btf = btpool.tile([P, TCH], BF16, tag="bt", name="btf")
                nc.vector.tensor_tensor(qkT[ft][:, tsl], ps[:], cos_sb[:, tsl], MULT)
                nc.vector.tensor_tensor(btf[:], ps[:], sin_sb[:, tsl], MULT)
                for blk in range(4):
                    a = blk * 32
                    c2 = a ^ 32
                    nc.gpsimd.dma_start(
                        qkT[ft][c2 : c2 + 32, tsl],
                        btf[a : a + 32, :],
                        accum_op=ADD,
                    )
            else:
                if tci == 0:
                    bt_live[ft] = btpool.tile([P, S], BF16, tag="btrow", name="btrow", bufs=2)
                btr = bt_live[ft]
                nc.vector.tensor_tensor(qkT[ft][:, tsl], ps[:], cos_sb[:, tsl], MULT)
                nc.vector.tensor_tensor(btr[:, tsl], ps[:], sin_sb[:, tsl], MULT)
                if tci == 3:
                    for blk in range(4):
                        a = blk * 32
                        c2 = a ^ 32  # partner half of the 64-row head block
                        nc.gpsimd.dma_start(
                            qkT[ft][c2 : c2 + 32, :],
                            btr[a : a + 32, :],
                            accum_op=ADD,
                        )

        # ---------------- attention chunk (software pipelined) ----------------
        LEAD = 4  # k-tiles of scores+exp emitted ahead of their AV matmuls

        def c_chunk(hp, qci, s_ps, av_ps, bc_ps, ktile_hook, prev_tail):
            # bc_ps: pool supplying the [128, 512] f32 PSUM tile for the
            # denominator-broadcast matmul (shared with proj/output pools).
            # Emits LEAD k-tiles of scores+exp first, then the previous
            # chunk's denominator tail, then the AV stream interleaved with
            # the remaining scores+exp -- so the ACT engine keeps streaming
            # exps across the avp-drain chunk boundary (av_ps bufs=1).
            qt = qkT[hp]
            ktt = qkT[4 + hp]
            h0, h1 = 2 * hp, 2 * hp + 1
            qsl = slice(qci * TCH, (qci + 1) * TCH)
            nkt = 4 * qci + 4
            avp = av_ps.tile([DH + 1, 2 * TCH], F32, tag="avp", name="avp")
            elive = {}

            def scores_exp(ki):
                ksl = slice(ki * P, (ki + 1) * P)
                diag = ki >= 4 * qci
                j = ki - 4 * qci
                off = j * P if diag else 0
                qlive = slice(qci * TCH + off, (qci + 1) * TCH)
                sp = s_ps.tile([P, 2 * TCH], F32, tag="sp", name="sp")
                mm(sp[:, off:TCH], ktt[0:64, ksl], qt[0:64, qlive], start=True, stop=True)
                mm(
                    sp[:, TCH + off : 2 * TCH],
                    ktt[64:128, ksl],
                    qt[64:128, qlive],
                    start=True,
                    stop=True,
                )
                if diag:
                    mm(
                        sp[:, off : off + P],
                        ident_sb[:],
                        mtri_sb[:],
                        start=False,
                        stop=True,
                        skip_group_check=True,
                    )
                    mm(
                        sp[:, TCH + off : TCH + off + P],
                        ident_sb[:],
                        mtri_sb[:],
                        start=False,
                        stop=True,
                        skip_group_check=True,
                    )
                e = epool.tile([P, 2 * TCH], BF16, tag="e", name="e")
                sp3 = sp[:].rearrange("p (h q) -> p h q", h=2)
                e3 = e[:].rearrange("p (h q) -> p h q", h=2)
                nc.scalar.activation(e3[:, :, off:], sp3[:, :, off:], EXP, scale=SCALE)
                elive[ki] = (e, off)

            def av(ki):
                e, off = elive.pop(ki)
                mm(
                    avp[:, off:TCH],
                    vbuf[:, ki, h0, :],
                    e[:, off:TCH],
                    start=(ki == 0),
                    stop=(ki == nkt - 1),
                    skip_group_check=True,
                )
                mm(
                    avp[:, TCH + off : 2 * TCH],
                    vbuf[:, ki, h1, :],
                    e[:, TCH + off : 2 * TCH],
                    start=(ki == 0),
                    stop=(ki == nkt - 1),
                    skip_group_check=True,
                )

            for ki in range(min(LEAD, nkt)):
                scores_exp(ki)
            if prev_tail is not None:
                prev_tail()
            for ki in range(nkt):
                av(ki)
                if ki + LEAD < nkt:
                    scores_exp(ki + LEAD)
                ktile_hook()
            # denominator chain head: drain PSUM row 64 to SBUF
            den0 = dpool.tile([1, 2 * TCH], BF16, tag="den", name="den0")
            nc.scalar.copy(den0[:], avp[DH : DH + 1, :])

            def tail():
                # two K=1 matmuls broadcast h0's denominators to partitions
                # 0:64 and h1's to 64:128 (block one-hot lhsT), accumulating
                # into one bank -- no partition-restack DMA needed.
                rb = bc_ps.tile([P, TCH], F32, tag="pj", name="rb")
                mm(
                    rb[:],
                    onesh_sb[0:1, 0:P],
                    den0[0:1, 0:TCH],
                    start=True,
                    stop=False,
                    skip_group_check=True,
                )
                mm(
                    rb[:],
                    onesh_sb[0:1, P : 2 * P],
                    den0[0:1, TCH : 2 * TCH],
                    start=False,
                    stop=True,
                    skip_group_check=True,
                )
                rec = rpool.tile([P, TCH], F32, tag="rec", name="rec")
                nc.vector.reciprocal_approx_fast(out=rec[:], in_=rb[:])
                nc.vector.tensor_tensor(
                    yT[hp][0:64, qsl], avp[0:DH, 0:TCH], rec[0:64, :], MULT
                )
                nc.vector.tensor_tensor(
                    yT[hp][64:128, qsl], avp[0:DH, TCH : 2 * TCH], rec[64:128, :], MULT
                )

            return tail

        # ---------------- output projection group ----------------
        def d_group(tti, jc, o_ps, osb, ring, drain="v"):
            tsl = slice(tti * P, (tti + 1) * P)
            jsl = slice(jc * TCH, (jc + 1) * TCH)
            op = o_ps.tile([P, TCH], F32, tag="pj", name="op")
            for cc in range(4):
                mm(
                    op[:],
                    yT[cc][:, tsl],
                    wo_sb[:, cc, jsl],
                    start=(cc == 0),
                    stop=(cc == 3),
                )
            ot = osb.tile([P, TCH], BF16, tag="ot", name="ot")
            if drain == "v":
                nc.vector.tensor_copy(ot[:], op[:])
            else:
                nc.scalar.copy(ot[:], op[:])
            ring.dma_start(outp.ap()[tsl, jsl], ot[:])

        # ---------------- prologue: A0 only (it gates the first exp) ----
        with ExitStack() as pro:
            pj0 = pro.enter_context(tc.tile_pool(name="pj0", bufs=2, space="PSUM"))
            for tci in range(4):
                for ft in (0, 4):
                    a_group(ft, tci, pj0, rope="dma_tci")
            for g in range(4):
                b_group(g, pj0)
            flush_a()
        nc.gpsimd.dma_start(wqk_sb[:, 2, :, :], wqkp[2].ap()[:, :, :])
        nc.gpsimd.dma_start(wqk_sb[:, 3, :, :], wqkp[3].ap()[:, :, :])

        # ---------------- main: C with interleaved proj groups ----------------
        with ExitStack() as cs:
            s_ps = cs.enter_context(tc.tile_pool(name="s_ps", bufs=2, space="PSUM"))
            av_ps = cs.enter_context(tc.tile_pool(name="av_ps", bufs=1, space="PSUM"))
            pj_stack = ExitStack()
            pjC = pj_stack.enter_context(
                tc.tile_pool(name="pjC", bufs=2, space="PSUM")
            )

            from collections import deque

            pendB = deque(range(4, 16))
            pendA = {
                h: deque((ft, tci) for ft in (h, 4 + h) for tci in range(4))
                for h in (1, 2, 3)
            }
            emittedB = [4]

            def emit_next():
                # A-groups first: their full-row swap-adds must clear the
                # gpsimd ring before the next head pair's scores.
                for h in (1, 2, 3):
                    if pendA[h]:
                        ft, tci = pendA[h].popleft()
                        a_group(ft, tci, pjC)
                        return True
                if pendB:
                    b_group(pendB.popleft(), pjC)
                    emittedB[0] += 1
                    return True
                return False

            def force_b(nkt):
                while emittedB[0] < nkt:
                    b_group(pendB.popleft(), pjC)
                    emittedB[0] += 1

            def force_a(h):
                while pendA[h]:
                    ft, tci = pendA[h].popleft()
                    a_group(ft, tci, pjC)
                flush_a()

            kglob = [0]

            def ktile_hook():
                kglob[0] += 1
                if kglob[0] % 2 == 0:
                    emit_next()

            tail = None
            for hp in range(3):
                for h in range(1, hp + 1):
                    force_a(h)
                for qci in range(4):
                    force_b(4 * qci + 4)
                    tail = c_chunk(hp, qci, s_ps, av_ps, pjC, ktile_hook, tail)

            # last head pair: free the proj bank, open output-proj PSUM
            force_a(3)
            tail()  # (2,3)'s denominator tail -- needs pjC, flush before close
            tail = None
            pj_stack.close()
            o_ps = cs.enter_context(tc.tile_pool(name="o_ps", bufs=2, space="PSUM"))
            osb = cs.enter_context(tc.tile_pool(name="osb", bufs=4))

            nohook = lambda: None  # noqa: E731
            d_next = [0]

            def emit_one_d():
                tti, jc = divmod(d_next[0], 2)
                d_group(tti, jc, o_ps, osb, nc.sync)
                d_next[0] += 1

            def d_hook(max_tti):
                def h():
                    if d_next[0] < max_tti * 2:
                        emit_one_d()
                return h

            tail = c_chunk(3, 0, s_ps, av_ps, o_ps, nohook, tail)
            tail = c_chunk(3, 1, s_ps, av_ps, o_ps, d_hook(2), tail)
            tail = c_chunk(3, 2, s_ps, av_ps, o_ps, d_hook(6), tail)
            tail = c_chunk(3, 3, s_ps, av_ps, o_ps, d_hook(12), tail)
            tail()

        # ---------------- tail of output projection ----------------
        with ExitStack() as ds:
            o2 = ds.enter_context(tc.tile_pool(name="o2", bufs=3, space="PSUM"))
            osb2 = ds.enter_context(tc.tile_pool(name="osb2", bufs=4))
            rings = [nc.sync, nc.scalar]
            while d_next[0] < n_kt * 2:
                tti, jc = divmod(d_next[0], 2)
                d_group(tti, jc, o2, osb2, rings[tti % 2], drain="vs"[jc])
                d_next[0] += 1


def _build():
    key = "nc_v2"
    if key in _CACHE:
        return _CACHE[key]
    nc = bacc.Bacc("TRN2", target_bir_lowering=False, debug=False, num_devices=8)
    xTp = [
        nc.dram_tensor(f"xTp{i}", [P, 8, TCH], BF16, kind="ExternalInput")
        for i in range(4)
    ]
    wqkp = [
        nc.dram_tensor(f"wqkp{g}", [P, 8, 2 * P], BF16, kind="ExternalInput")
        for g in range(4)
    ]
    wvp = nc.dram_tensor("wvp", [P, 8, 512], BF16, kind="ExternalInput")
    wop = nc.dram_tensor("wop", [P, 4, 1024], BF16, kind="ExternalInput")
    cosp = nc.dram_tensor("cosp", [P, S], F32, kind="ExternalInput")
    sinp = nc.dram_tensor("sinp", [P, S], F32, kind="ExternalInput")
    mtri = nc.dram_tensor("mtri", [P, P], BF16, kind="ExternalInput")
    ident = nc.dram_tensor("ident", [P, P], BF16, kind="ExternalInput")
    onesh = nc.dram_tensor("onesh", [1, 2 * P], BF16, kind="ExternalInput")
    permP = nc.dram_tensor("permP", [P, P], BF16, kind="ExternalInput")
    outp = nc.dram_tensor("outp", [S, D], BF16, kind="ExternalOutput")
    with tile.TileContext(nc) as tc:
        _emit(nc, tc, xTp, wqkp, wvp, wop, cosp, sinp, mtri, ident, onesh, permP, outp)
    nc.compile()
    _CACHE[key] = nc
    return nc


def host_inputs(x, wqkv, wo, token_positions):
    """Build the 8 per-core input maps (host-side sharding / layout prep)."""
    import ml_dtypes

    x = np.asarray(x, dtype=np.float32)
    wqkv = np.asarray(wqkv, dtype=np.float32)
    wo = np.asarray(wo, dtype=np.float32)
    pos = np.asarray(token_positions).astype(np.float32)

    d_model = x.shape[2]
    wq, wk, wv = wqkv[0:d_model], wqkv[d_model : 2 * d_model], wqkv[2 * d_model :]

    inv = np.float32(ROPE_THETA) ** (
        -np.arange(0, DH, 2, dtype=np.float32) / np.float32(DH)
    )  # [32]
    ang = pos[None, :] * inv[:, None]  # [32, S]
    cos32 = np.cos(ang).astype(np.float32)
    sin32 = np.sin(ang).astype(np.float32)
    cosp = np.ascontiguousarray(np.tile(cos32, (4, 1)))  # [128, S]
    sinp = np.ascontiguousarray(
        np.tile(np.concatenate([sin32, -sin32], axis=0), (2, 1))
    )  # [128, S]

    a = np.arange(P)
    mtri = np.where(a[:, None] > a[None, :], np.float32(NEG), np.float32(0.0))
    mtri = mtri.astype(ml_dtypes.bfloat16)
    ident = np.eye(P, dtype=ml_dtypes.bfloat16)
    onesh = np.zeros((1, 2 * P), np.float32)
    onesh[0, 0:64] = 1.0
    onesh[0, P + 64 : 2 * P] = 1.0
    onesh = onesh.astype(ml_dtypes.bfloat16)
    permP = (a[:, None] == (a[None, :] ^ 32)).astype(ml_dtypes.bfloat16)

    perm64 = np.concatenate([np.arange(0, DH, 2), np.arange(1, DH, 2)])

    def pmajor(mat, eo):
        # [eo*128, f] -> [128, eo, f]
        return np.ascontiguousarray(
            mat.reshape(eo, P, mat.shape[1]).transpose(1, 0, 2)
        ).astype(ml_dtypes.bfloat16)

    in_maps = []
    for ci in range(8):
        bi, hg = divmod(ci, 2)
        xT = np.ascontiguousarray(x[bi].T)  # [1024, 2048]
        xTr = pmajor(xT, 8)  # [128, 8, 2048]
        rows = []
        for blk in (wq, wk):
            for h in range(hg * NH_CORE, (hg + 1) * NH_CORE):
                rows.append(blk[h * DH : (h + 1) * DH][perm64])
        wqkT = np.ascontiguousarray(np.concatenate(rows, axis=0).T)  # [1024, 1024]
        wvT = np.ascontiguousarray(wv[hg * 512 : (hg + 1) * 512].T)  # [1024, 512]
        woT = np.ascontiguousarray(wo[:, hg * 512 : (hg + 1) * 512].T)  # [512, 1024]
        m = {
            "wvp": pmajor(wvT, 8),
            "wop": pmajor(woT, 4),
            "cosp": cosp,
            "sinp": sinp,
            "mtri": mtri,
            "ident": ident,
            "onesh": onesh,
            "permP": permP,
        }
        for g in range(4):
            wg = np.concatenate(
                [
                    wqkT[:, g * P : (g + 1) * P],
                    wqkT[:, (4 + g) * P : (5 + g) * P],
                ],
                axis=1,
            )  # [1024, 256]
            m[f"wqkp{g}"] = pmajor(wg, 8)
        for i in range(4):
            m[f"xTp{i}"] = np.ascontiguousarray(xTr[:, :, i * TCH : (i + 1) * TCH])
        in_maps.append(m)
    return in_maps


def _install_ntff_hook():
    """Recreate the antenv.axon_hooks NTFF profile hook this image lacks
    (same ctypes shim trn_agent_boot would register). Dev/profiling only."""
    import contextlib
    import ctypes
    import os
    import types

    try:
        import antenv.axon_hooks  # noqa: F401

        return
    except ImportError:
        pass
    so_path = "/opt/axon/libaxon_pjrt.so"
    if not os.path.exists(so_path):
        return
    lib = ctypes.CDLL(so_path)
    if not hasattr(lib, "axon_start_nrt_profile"):
        return
    lib.axon_start_nrt_profile.argtypes = [
        ctypes.POINTER(ctypes.c_int64),
        ctypes.c_size_t,
    ]
    lib.axon_start_nrt_profile.restype = ctypes.c_int64
    lib.axon_stop_nrt_profile.argtypes = [ctypes.c_char_p]
    lib.axon_stop_nrt_profile.restype = ctypes.c_int64

    @contextlib.contextmanager
    def _hook(output_dir, device_ids):
        import jax

        jax.devices()
        if device_ids:
            ids = (ctypes.c_int64 * len(device_ids))(*device_ids)
            rc = lib.axon_start_nrt_profile(ids, len(device_ids))
        else:
            rc = lib.axon_start_nrt_profile(None, 0)
        if rc != 0:
            raise RuntimeError(f"axon_start_nrt_profile rc={rc}")
        try:
            yield
        finally:
            n = lib.axon_stop_nrt_profile(str(output_dir).encode())
            if n < 0:
                raise RuntimeError(f"axon_stop_nrt_profile rc={n}")

    import antenv
    from concourse import bass_utils as _bu

    _bu.upload_artifacts = lambda d: d  # no bucket access in this container
    mod = types.ModuleType("antenv.axon_hooks")
    mod.get_axon_ntff_profile_hook = lambda: _hook
    mod.set_axon_ntff_profile_hook = lambda h: None
    sys.modules["antenv.axon_hooks"] = mod
    antenv.axon_hooks = mod


def kernel(x, wqkv, wo, token_positions, trace=False):
    if trace:
        _install_ntff_hook()
    nc = _build()
    in_maps = host_inputs(x, wqkv, wo, token_positions)
    res = run_bass_kernel_spmd(nc, in_maps, core_ids=list(range(8)), trace=trace)
    parts = [np.asarray(res.results[ci]["outp"]).astype(np.float32) for ci in range(8)]
    out = np.stack([parts[2 * bi] + parts[2 * bi + 1] for bi in range(B)], axis=0)
    if trace:
        kernel.last_result = res
    return out
